# revision 6
# baseline (speedup 1.0000x reference)
"""Causal self-attention on 8 TRN2 NeuronCores.

Sharding: core_id = 2*b + g  (b = batch 0..3, g = head-group 0..1, 8 heads each).
Each core computes qkv for its 8 heads, causal flash-style attention, and a
partial projection (its 512 channels x full w_proj rows). Host sums the two
partials per batch.

Layout strategy (everything transposed so no on-device transposes are needed):
  - x^T [C, T] per batch (host pre-transposed, bf16)
  - Q^T, K^T computed as W^T @ x^T  -> [512, 2048] (channel on partitions)
  - V computed directly as x @ W_v  -> [2048, 512] (token on partitions),
    stored with a ones column per head (V' = [V_h | 1]) so the attention AV
    matmul also produces the softmax row-sums.
  - S^T = K @ Q^T per 128-token j-chunk, both heads of a pair row-tiled into
    one PSUM tile; exp on ACT; causal zeroing via affine_select on GPSIMD.
  - O^T accumulated in PSUM, normalized with reciprocal+partition_broadcast.
  - proj consumes O^T as the stationary matmul operand.
"""

import numpy as np
import ml_dtypes

import concourse.bass as bass
import concourse.tile as tile
from concourse import bacc, mybir
from concourse.bass_utils import run_bass_kernel_spmd

BF16 = ml_dtypes.bfloat16

B, T, C = 4, 2048, 1024
H = 16               # total heads
D = C // H           # 64
HG = 8               # heads per core (head-group)
CL = HG * D          # 512 local channels
N_CORES = 8
SCALE = 1.0 / float(np.sqrt(D))

NCC = C // 128       # 8 c-chunks
NT4 = T // 512       # 4 t-tiles of 512
NT16 = T // 128      # 16 t-chunks of 128

_CACHE = {}


def _emit_body(nc, pools, tensors, use_bias, rep):
    dt = mybir.dt
    psum_mm, psum_s, psum_o, pwork, ywork, norm = pools
    (xt, wqk, wv, wp, qkt, vps, otp, y_d, ones_row, bqk_sb, bv_sb) = tensors

    # ---- phase 1a: Q^T, K^T  (channel-major, bias is per-partition) ----
    for mc in range(8):
        for tt in range(NT4):
            ps = psum_mm.tile([128, 512], dt.float32, tag="mm",
                              name=f"mm_qk_{rep}_{mc}_{tt}")
            for cc in range(NCC):
                nc.tensor.matmul(
                    ps[:],
                    wqk[cc][:, mc * 128:(mc + 1) * 128],
                    xt[cc][:, tt * 512:(tt + 1) * 512],
                    start=(cc == 0), stop=(cc == NCC - 1 and not use_bias),
                )
            if use_bias:
                nc.tensor.matmul(
                    ps[:],
                    bqk_sb[:, mc * 128:(mc + 1) * 128],
                    ones_row[:, tt * 512:(tt + 1) * 512],
                    start=False, stop=True,
                )
            nc.vector.tensor_copy(qkt[mc][:, tt * 512:(tt + 1) * 512], ps[:])

    # ---- phase 1b: V' (token-major) ----
    for t16 in range(NT16):
        ps = psum_mm.tile([128, 512], dt.float32, tag="mm", name=f"mm_v_{rep}_{t16}")
        for cc in range(NCC):
            nc.tensor.matmul(
                ps[:],
                xt[cc][:, t16 * 128:(t16 + 1) * 128],
                wv[cc][:],
                start=(cc == 0), stop=(cc == NCC - 1 and not use_bias),
            )
        if use_bias:
            nc.tensor.matmul(
                ps[:],
                ones_row[:, t16 * 128:(t16 + 1) * 128],
                bv_sb[:],
                start=False, stop=True,
            )
        vt = vps[t16]
        nc.vector.memset(vt[:], 1.0)
        nc.vector.tensor_copy(
            vt[:].rearrange("p (h e) -> p h e", e=D + 1)[:, :, 0:D],
            ps[:].rearrange("p (h d) -> p h d", d=D),
        )

    # ---- phase 2: attention + phase 3 proj, q-tile major ----
    for qt4 in range(NT4):
        q0 = qt4 * 512
        nj = 4 * (qt4 + 1)
        for hp in range(4):
            qts, kts = qkt[hp], qkt[4 + hp]
            o_ps = []
            for hi in range(2):
                o_ps.append(psum_o.tile([D + 1, 512], dt.float32, tag="o",
                                        name=f"o_{rep}_{qt4}_{hp}_{hi}"))
            for jc in range(nj):
                j0 = jc * 128
                s_pair = psum_s.tile([128, 1024], dt.float32, tag="s",
                                     name=f"s_{rep}_{qt4}_{hp}_{jc}")
                for hi in range(2):
                    nc.tensor.matmul(
                        s_pair[:, hi * 512:(hi + 1) * 512],
                        kts[hi * D:(hi + 1) * D, j0:j0 + 128],
                        qts[hi * D:(hi + 1) * D, q0:q0 + 512],
                        start=True, stop=True,
                    )
                p_pair = pwork.tile([128, 1024], dt.bfloat16, tag="p",
                                    name=f"p_{rep}_{qt4}_{hp}_{jc}")
                nc.scalar.activation(
                    p_pair[:], s_pair[:],
                    mybir.ActivationFunctionType.Exp, scale=SCALE)
                off = j0 - q0
                if off > -128:
                    # keep where q_global >= j_global:
                    #   (q0+qi) - (j0+jj) >= 0  ->  qi - jj - off >= 0
                    nc.gpsimd.affine_select(
                        out=p_pair[:].rearrange("p (h q) -> p h q", h=2),
                        in_=p_pair[:].rearrange("p (h q) -> p h q", h=2),
                        compare_op=mybir.AluOpType.is_ge,
                        fill=0.0, base=-off,
                        pattern=[[0, 2], [1, 512]],
                        channel_multiplier=-1,
                    )
                for hi in range(2):
                    h = 2 * hp + hi
                    nc.tensor.matmul(
                        o_ps[hi][:],
                        vps[jc][:, h * (D + 1):(h + 1) * (D + 1)],
                        p_pair[:, hi * 512:(hi + 1) * 512],
                        start=(jc == 0), stop=(jc == nj - 1),
                    )
            for hi in range(2):
                recip = norm.tile([1, 512], dt.float32, tag="recip",
                                  name=f"recip_{rep}_{qt4}_{hp}_{hi}")
                nc.vector.reciprocal(recip[:], o_ps[hi][D:D + 1, :])
                bcast = norm.tile([D, 512], dt.float32, tag="bcast",
                                  name=f"bcast_{rep}_{qt4}_{hp}_{hi}")
                nc.gpsimd.partition_broadcast(bcast[:], recip[:])
                nc.vector.tensor_mul(
                    otp[hp][hi * D:(hi + 1) * D, q0:q0 + 512],
                    o_ps[hi][0:D, :], bcast[:])

        # ---- proj for this q range ----
        for qc in range(qt4 * 4, qt4 * 4 + 4):
            for nt in range(2):
                ps = psum_mm.tile([128, 512], dt.float32, tag="mm",
                                  name=f"mm_y_{rep}_{qc}_{nt}")
                for hp in range(4):
                    nc.tensor.matmul(
                        ps[:],
                        otp[hp][:, qc * 128:(qc + 1) * 128],
                        wp[hp][:, nt * 512:(nt + 1) * 512],
                        start=(hp == 0), stop=(hp == 3),
                    )
                y_sb = ywork.tile([128, 512], dt.float32, tag="y",
                                  name=f"y_{rep}_{qc}_{nt}")
                nc.vector.tensor_copy(y_sb[:], ps[:])
                nc.sync.dma_start(
                    y_d[qc * 128:(qc + 1) * 128, nt * 512:(nt + 1) * 512],
                    y_sb[:])


def _build(use_bias: bool, reps: int = 1):
    nc = bacc.Bacc("TRN2", target_bir_lowering=False, debug=False,
                   num_devices=N_CORES)
    dt = mybir.dt

    xt_d = nc.dram_tensor("xt", [C, T], dt.bfloat16, kind="ExternalInput").ap()
    wqk_d = nc.dram_tensor("wqk", [C, 2 * CL], dt.bfloat16, kind="ExternalInput").ap()
    wv_d = nc.dram_tensor("wv", [C, CL], dt.bfloat16, kind="ExternalInput").ap()
    wp_d = nc.dram_tensor("wp", [CL, C], dt.bfloat16, kind="ExternalInput").ap()
    bqk_d = bv_d = None
    if use_bias:
        bqk_d = nc.dram_tensor("bqk", [2 * CL], dt.bfloat16, kind="ExternalInput").ap()
        bv_d = nc.dram_tensor("bv", [CL], dt.bfloat16, kind="ExternalInput").ap()
    y_d = nc.dram_tensor("y", [T, C], dt.float32, kind="ExternalOutput").ap()

    with tile.TileContext(nc) as tc:
        with (
            tc.tile_pool(name="const", bufs=1) as const,
            tc.tile_pool(name="psum_mm", bufs=2, space="PSUM") as psum_mm,
            tc.tile_pool(name="psum_s", bufs=2, space="PSUM") as psum_s,
            tc.tile_pool(name="psum_o", bufs=2, space="PSUM") as psum_o,
            tc.tile_pool(name="pwork", bufs=3) as pwork,
            tc.tile_pool(name="ywork", bufs=3) as ywork,
            tc.tile_pool(name="norm", bufs=4) as norm,
        ):
            # ---- persistent SBUF inputs ----
            xt = []
            for cc in range(NCC):
                t = const.tile([128, T], dt.bfloat16, tag=f"xt{cc}", name=f"xt{cc}")
                nc.sync.dma_start(t[:], xt_d[cc * 128:(cc + 1) * 128, :])
                xt.append(t)
            wqk = []
            for cc in range(NCC):
                t = const.tile([128, 2 * CL], dt.bfloat16, tag=f"wqk{cc}",
                               name=f"wqk{cc}")
                nc.sync.dma_start(t[:], wqk_d[cc * 128:(cc + 1) * 128, :])
                wqk.append(t)
            wv = []
            for cc in range(NCC):
                t = const.tile([128, CL], dt.bfloat16, tag=f"wv{cc}", name=f"wv{cc}")
                nc.sync.dma_start(t[:], wv_d[cc * 128:(cc + 1) * 128, :])
                wv.append(t)
            wp = []
            for hp in range(4):
                t = const.tile([128, C], dt.bfloat16, tag=f"wp{hp}", name=f"wp{hp}")
                nc.sync.dma_start(t[:], wp_d[hp * 128:(hp + 1) * 128, :])
                wp.append(t)
            ones_row = bqk_sb = bv_sb = None
            if use_bias:
                ones_row = const.tile([1, T], dt.bfloat16, tag="ones_row",
                                      name="ones_row")
                nc.vector.memset(ones_row[:], 1.0)
                bqk_sb = const.tile([1, 2 * CL], dt.bfloat16, tag="bqk", name="bqk_sb")
                nc.sync.dma_start(bqk_sb[:], bqk_d[:].rearrange("n -> 1 n"))
                bv_sb = const.tile([1, CL], dt.bfloat16, tag="bv", name="bv_sb")
                nc.sync.dma_start(bv_sb[:], bv_d[:].rearrange("n -> 1 n"))

            # persistent intermediate tensors
            qkt = []   # 8 tiles [128, T]: 0..3 = Q^T head-pairs, 4..7 = K^T
            for i in range(8):
                qkt.append(const.tile([128, T], dt.bfloat16, tag=f"qkt{i}",
                                      name=f"qkt{i}"))
            vps = []   # 16 tiles [128, 8*65]: V' per t-chunk
            for i in range(NT16):
                vps.append(const.tile([128, HG * (D + 1)], dt.bfloat16,
                                      tag=f"vp{i}", name=f"vp{i}"))
            otp = []   # 4 tiles [128, T]: O^T head-pairs
            for hp in range(4):
                otp.append(const.tile([128, T], dt.bfloat16, tag=f"otp{hp}",
                                      name=f"otp{hp}"))

            pools = (psum_mm, psum_s, psum_o, pwork, ywork, norm)
            tensors = (xt, wqk, wv, wp, qkt, vps, otp, y_d, ones_row, bqk_sb, bv_sb)
            for rep in range(reps):
                _emit_body(nc, pools, tensors, use_bias, rep)

    nc.compile()
    return nc


def _get_nc(use_bias: bool, reps: int = 1):
    key = (use_bias, reps)
    if key not in _CACHE:
        _CACHE[key] = _build(use_bias, reps)
    return _CACHE[key]


def _make_in_maps(x, w_qkv, b_qkv, w_proj, use_bias):
    xts = [np.ascontiguousarray(x[b].T).astype(BF16) for b in range(B)]
    parts = []
    for g in range(2):
        sl = slice(g * CL, (g + 1) * CL)
        wqk = np.ascontiguousarray(np.concatenate(
            [w_qkv[:, 0:C][:, sl], w_qkv[:, C:2 * C][:, sl]], axis=1)).astype(BF16)
        wv = np.ascontiguousarray(w_qkv[:, 2 * C:3 * C][:, sl]).astype(BF16)
        wp = np.ascontiguousarray(w_proj[sl, :]).astype(BF16)
        d = {"wqk": wqk, "wv": wv, "wp": wp}
        if use_bias:
            d["bqk"] = np.ascontiguousarray(np.concatenate(
                [b_qkv[0:C][sl], b_qkv[C:2 * C][sl]])).astype(BF16)
            d["bv"] = np.ascontiguousarray(b_qkv[2 * C:3 * C][sl]).astype(BF16)
        parts.append(d)
    return [dict(parts[core % 2], xt=xts[core // 2]) for core in range(N_CORES)]


def kernel(x, w_qkv, b_qkv, w_proj, b_proj):
    x = np.asarray(x, dtype=np.float32)
    w_qkv = np.asarray(w_qkv, dtype=np.float32)
    b_qkv = np.asarray(b_qkv, dtype=np.float32)
    w_proj = np.asarray(w_proj, dtype=np.float32)
    b_proj = np.asarray(b_proj, dtype=np.float32)

    use_bias = bool(np.any(b_qkv))
    nc = _get_nc(use_bias)
    in_maps = _make_in_maps(x, w_qkv, b_qkv, w_proj, use_bias)

    res = run_bass_kernel_spmd(nc, in_maps, list(range(N_CORES)))
    y = np.empty((B, T, C), dtype=np.float32)
    for b in range(B):
        y[b] = res.results[2 * b]["y"] + res.results[2 * b + 1]["y"]
    if np.any(b_proj):
        y += b_proj[None, None, :]
    return y


# revision 15
# speedup vs baseline: 586.6945x; 586.6945x over previous
"""Causal self-attention on 8 TRN2 NeuronCores.

Sharding: core_id = 2*b + g  (b = batch 0..3, g = head-group 0..1, 8 heads each).
Each core computes qkv for its 8 heads, causal flash-style attention, and a
partial projection (its 512 channels x full w_proj rows). Host sums the two
partials per batch.

Layout strategy (everything transposed so no on-device transposes are needed):
  - x^T [C, T] per batch (host pre-transposed, bf16)
  - Q^T, K^T computed as W^T @ x^T  -> [512, 2048] (channel on partitions)
  - V computed directly as x @ W_v  -> [2048, 512] (token on partitions),
    stored with a ones column per head (V' = [V_h | 1]) so the attention AV
    matmul also produces the softmax row-sums.
  - S^T = K @ Q^T per 128-token j-chunk, both heads of a pair row-tiled into
    one PSUM tile; exp on ACT; causal zeroing via affine_select on GPSIMD.
  - O^T accumulated in PSUM, normalized with reciprocal+partition_broadcast.
  - proj consumes O^T as the stationary matmul operand.
"""

import numpy as np
import ml_dtypes

import concourse.bass as bass
import concourse.tile as tile
from concourse import bacc, mybir
from concourse.bass_utils import run_bass_kernel_spmd

BF16 = ml_dtypes.bfloat16

B, T, C = 4, 2048, 1024
H = 16               # total heads
D = C // H           # 64
HG = 8               # heads per core (head-group)
CL = HG * D          # 512 local channels
N_CORES = 8
SCALE = 1.0 / float(np.sqrt(D))

NCC = C // 128       # 8 c-chunks
NT4 = T // 512       # 4 t-tiles of 512
NT16 = T // 128      # 16 t-chunks of 128

_CACHE = {}


def _emit_body(nc, pools, tensors, use_bias, rep):
    dt = mybir.dt
    psum_s, psum_o, pwork, ywork, norm = pools
    (xt, wqk, wv, wp, qkt, vps, otp, y_d, ones_row, bqk_sb, bv_sb) = tensors

    # ---- phase 1a helper: Q^T or K^T for one 128-channel chunk ----
    def emit_qk(mc, tts=range(NT4)):
        for tt in tts:
            ps = psum_s.tile([128, 512], dt.float32, tag="s",
                              name=f"mm_qk_{rep}_{mc}_{tt}")
            for cc in range(NCC):
                nc.tensor.matmul(
                    ps[:],
                    wqk[cc][:, mc * 128:(mc + 1) * 128],
                    xt[cc][:, tt * 512:(tt + 1) * 512],
                    start=(cc == 0), stop=(cc == NCC - 1 and not use_bias),
                )
            if use_bias:
                nc.tensor.matmul(
                    ps[:],
                    bqk_sb[:, mc * 128:(mc + 1) * 128],
                    ones_row[:, tt * 512:(tt + 1) * 512],
                    start=False, stop=True,
                )
            nc.vector.tensor_copy(qkt[mc][:, tt * 512:(tt + 1) * 512], ps[:])

    def emit_v(t16):
        ps = psum_s.tile([128, 512], dt.float32, tag="s", name=f"mm_v_{rep}_{t16}")
        for cc in range(NCC):
            nc.tensor.matmul(
                ps[:],
                xt[cc][:, t16 * 128:(t16 + 1) * 128],
                wv[cc][:],
                start=(cc == 0), stop=(cc == NCC - 1 and not use_bias),
            )
        if use_bias:
            nc.tensor.matmul(
                ps[:],
                ones_row[:, t16 * 128:(t16 + 1) * 128],
                bv_sb[:],
                start=False, stop=True,
            )
        vt = vps[t16]
        nc.vector.tensor_copy(
            vt[:].rearrange("p (h e) -> p h e", e=D + 1)[:, :, 0:D],
            ps[:].rearrange("p (h d) -> p h d", d=D),
        )

    def emit_attn(hp, qt4):
        q0 = qt4 * 512
        nj = 4 * (qt4 + 1)
        qts, kts = qkt[hp], qkt[4 + hp]
        o_ps = []
        for hi in range(2):
            o_ps.append(psum_o.tile([D + 1, 512], dt.float32, tag="o", bufs=2,
                                    name=f"o_{rep}_{qt4}_{hp}_{hi}"))
        for jc in range(nj):
            j0 = jc * 128
            off = j0 - q0
            # diagonal blocks: only columns q >= j0 can be unmasked
            c0 = max(0, off)        # first useful column in this q-tile
            w = 512 - c0            # columns computed
            s_pair = psum_s.tile([128, 1024], dt.float32, tag="s",
                                 name=f"s_{rep}_{qt4}_{hp}_{jc}")
            for hi in range(2):
                nc.tensor.matmul(
                    s_pair[:, hi * 512 + c0:(hi + 1) * 512],
                    kts[hi * D:(hi + 1) * D, j0:j0 + 128],
                    qts[hi * D:(hi + 1) * D, q0 + c0:q0 + 512],
                    start=True, stop=True,
                )
            p_pair = pwork.tile([128, 1024], dt.bfloat16, tag="p",
                                name=f"p_{rep}_{qt4}_{hp}_{jc}")
            pv = p_pair[:].rearrange("p (h q) -> p h q", h=2)[:, :, c0:512]
            nc.scalar.activation(
                pv,
                s_pair[:].rearrange("p (h q) -> p h q", h=2)[:, :, c0:512],
                mybir.ActivationFunctionType.Exp, scale=SCALE)
            if off > -128:
                # keep where q_global >= j_global; in the clipped view the
                # column index is qi' = qi - c0, so keep iff qi' >= jj.
                nc.gpsimd.affine_select(
                    out=pv, in_=pv,
                    compare_op=mybir.AluOpType.is_ge,
                    fill=0.0, base=0,
                    pattern=[[0, 2], [1, w]],
                    channel_multiplier=-1,
                )
            for hi in range(2):
                h = 2 * hp + hi
                nc.tensor.matmul(
                    o_ps[hi][:, c0:512],
                    vps[jc][:, h * (D + 1):(h + 1) * (D + 1)],
                    p_pair[:, hi * 512 + c0:(hi + 1) * 512],
                    start=(jc == 0), stop=(jc == nj - 1),
                )
        for hi in range(2):
            recip = norm.tile([1, 512], dt.float32, tag="recip",
                              name=f"recip_{rep}_{qt4}_{hp}_{hi}")
            nc.vector.reciprocal(recip[:], o_ps[hi][D:D + 1, :])
            bcast = norm.tile([D, 512], dt.float32, tag="bcast",
                              name=f"bcast_{rep}_{qt4}_{hp}_{hi}")
            nc.gpsimd.partition_broadcast(bcast[:], recip[:])
            nc.vector.tensor_mul(
                otp[hp][hi * D:(hi + 1) * D, q0:q0 + 512],
                o_ps[hi][0:D, :], bcast[:])

    def emit_proj(qt4):
        for qc in range(qt4 * 4, qt4 * 4 + 4):
            for nt in range(2):
                ps = psum_s.tile([128, 512], dt.float32, tag="s",
                                  name=f"mm_y_{rep}_{qc}_{nt}")
                for hp in range(4):
                    nc.tensor.matmul(
                        ps[:],
                        otp[hp][:, qc * 128:(qc + 1) * 128],
                        wp[hp][:, nt * 512:(nt + 1) * 512],
                        start=(hp == 0), stop=(hp == 3),
                    )
                y_sb = ywork.tile([128, 512], dt.float32, tag="y",
                                  name=f"y_{rep}_{qc}_{nt}")
                nc.vector.tensor_copy(y_sb[:], ps[:])
                nc.sync.dma_start(
                    y_d[qc * 128:(qc + 1) * 128, nt * 512:(nt + 1) * 512],
                    y_sb[:])

    # Emission order tuned for overlap: get head-pair 0 ready fast so the
    # ACT-bound attention starts early, then stream the remaining QKV work
    # into PE gaps. Proj for a q-range is emitted as soon as the last
    # head-pair has produced it (attention loops qt4-major inside hp).
    # Lead-in: Q,K (first t-tile) for ALL head-pairs + V for the first
    # q-tile, then sweep q-tiles with head-pairs inner — every attention
    # unit's dependencies are ready well before ACT reaches it, and the
    # next t-tile's QKV streams into PE gaps during each sweep.
    for mc in range(8):
        emit_qk(mc, [0])
    for t16 in range(4):
        emit_v(t16)
    for qt4 in range(NT4):
        for hp in range(4):
            emit_attn(hp, qt4)
            if qt4 < NT4 - 1:
                # prefetch next q-tile's QKV in small per-unit chunks
                emit_qk(2 * hp, [qt4 + 1])
                emit_qk(2 * hp + 1, [qt4 + 1])
                if hp < 2:
                    emit_v(4 * qt4 + 4 + 2 * hp)
                    emit_v(4 * qt4 + 5 + 2 * hp)
        emit_proj(qt4)


def _build(use_bias: bool, reps: int = 1):
    nc = bacc.Bacc("TRN2", target_bir_lowering=False, debug=False,
                   num_devices=N_CORES)
    dt = mybir.dt

    xt_d = nc.dram_tensor("xt", [C, T], dt.bfloat16, kind="ExternalInput").ap()
    wqk_d = nc.dram_tensor("wqk", [C, 2 * CL], dt.bfloat16, kind="ExternalInput").ap()
    wv_d = nc.dram_tensor("wv", [C, CL], dt.bfloat16, kind="ExternalInput").ap()
    wp_d = nc.dram_tensor("wp", [CL, C], dt.bfloat16, kind="ExternalInput").ap()
    bqk_d = bv_d = None
    if use_bias:
        bqk_d = nc.dram_tensor("bqk", [2 * CL], dt.bfloat16, kind="ExternalInput").ap()
        bv_d = nc.dram_tensor("bv", [CL], dt.bfloat16, kind="ExternalInput").ap()
    y_d = nc.dram_tensor("y", [T, C], dt.float32, kind="ExternalOutput").ap()

    with tile.TileContext(nc) as tc:
        with (
            tc.tile_pool(name="const", bufs=1) as const,
            tc.tile_pool(name="psum_s", bufs=3, space="PSUM") as psum_s,
            tc.tile_pool(name="psum_o", bufs=2, space="PSUM") as psum_o,
            tc.tile_pool(name="pwork", bufs=8) as pwork,
            tc.tile_pool(name="ywork", bufs=4) as ywork,
            tc.tile_pool(name="norm", bufs=6) as norm,
        ):
            # ---- persistent SBUF inputs ----
            # Lead-in DMAs first: the slices attention q-tile 0 needs, so
            # compute starts ~4us in instead of after the full ~8MB load.
            xt = [const.tile([128, T], dt.bfloat16, tag=f"xt{cc}", name=f"xt{cc}")
                  for cc in range(NCC)]
            wqk = [const.tile([128, 2 * CL], dt.bfloat16, tag=f"wqk{cc}",
                              name=f"wqk{cc}") for cc in range(NCC)]
            wv = [const.tile([128, CL], dt.bfloat16, tag=f"wv{cc}", name=f"wv{cc}")
                  for cc in range(NCC)]
            wp = [const.tile([128, C], dt.bfloat16, tag=f"wp{hp}", name=f"wp{hp}")
                  for hp in range(4)]
            for cc in range(NCC):
                r = slice(cc * 128, (cc + 1) * 128)
                nc.sync.dma_start(xt[cc][:, 0:512], xt_d[r, 0:512])
                nc.sync.dma_start(wqk[cc][:, 0:128], wqk_d[r, 0:128])
                nc.sync.dma_start(wqk[cc][:, CL:CL + 128], wqk_d[r, CL:CL + 128])
                nc.sync.dma_start(wv[cc][:], wv_d[r, :])
            for cc in range(NCC):
                r = slice(cc * 128, (cc + 1) * 128)
                nc.sync.dma_start(xt[cc][:, 512:T], xt_d[r, 512:T])
                nc.sync.dma_start(wqk[cc][:, 128:CL], wqk_d[r, 128:CL])
                nc.sync.dma_start(wqk[cc][:, CL + 128:2 * CL],
                                  wqk_d[r, CL + 128:2 * CL])
            for hp in range(4):
                nc.sync.dma_start(wp[hp][:], wp_d[hp * 128:(hp + 1) * 128, :])
            ones_row = bqk_sb = bv_sb = None
            if use_bias:
                ones_row = const.tile([1, T], dt.bfloat16, tag="ones_row",
                                      name="ones_row")
                nc.vector.memset(ones_row[:], 1.0)
                bqk_sb = const.tile([1, 2 * CL], dt.bfloat16, tag="bqk", name="bqk_sb")
                nc.sync.dma_start(bqk_sb[:], bqk_d[:].rearrange("n -> 1 n"))
                bv_sb = const.tile([1, CL], dt.bfloat16, tag="bv", name="bv_sb")
                nc.sync.dma_start(bv_sb[:], bv_d[:].rearrange("n -> 1 n"))

            # persistent intermediate tensors
            qkt = []   # 8 tiles [128, T]: 0..3 = Q^T head-pairs, 4..7 = K^T
            for i in range(8):
                qkt.append(const.tile([128, T], dt.bfloat16, tag=f"qkt{i}",
                                      name=f"qkt{i}"))
            vps = []   # 16 tiles [128, 8*65]: V' per t-chunk
            for i in range(NT16):
                vt = const.tile([128, HG * (D + 1)], dt.bfloat16,
                                tag=f"vp{i}", name=f"vp{i}")
                # ones column per head (col 64 of each 65-wide group)
                nc.vector.memset(
                    vt[:].rearrange("p (h e) -> p h e", e=D + 1)[:, :, D:D + 1], 1.0)
                vps.append(vt)
            otp = []   # 4 tiles [128, T]: O^T head-pairs
            for hp in range(4):
                otp.append(const.tile([128, T], dt.bfloat16, tag=f"otp{hp}",
                                      name=f"otp{hp}"))

            pools = (psum_s, psum_o, pwork, ywork, norm)
            tensors = (xt, wqk, wv, wp, qkt, vps, otp, y_d, ones_row, bqk_sb, bv_sb)
            for rep in range(reps):
                _emit_body(nc, pools, tensors, use_bias, rep)

    nc.compile()
    return nc


def _get_nc(use_bias: bool, reps: int = 1):
    key = (use_bias, reps)
    if key not in _CACHE:
        _CACHE[key] = _build(use_bias, reps)
    return _CACHE[key]


def _make_in_maps(x, w_qkv, b_qkv, w_proj, use_bias):
    xts = [np.ascontiguousarray(x[b].T).astype(BF16) for b in range(B)]
    parts = []
    for g in range(2):
        sl = slice(g * CL, (g + 1) * CL)
        wqk = np.ascontiguousarray(np.concatenate(
            [w_qkv[:, 0:C][:, sl], w_qkv[:, C:2 * C][:, sl]], axis=1)).astype(BF16)
        wv = np.ascontiguousarray(w_qkv[:, 2 * C:3 * C][:, sl]).astype(BF16)
        wp = np.ascontiguousarray(w_proj[sl, :]).astype(BF16)
        d = {"wqk": wqk, "wv": wv, "wp": wp}
        if use_bias:
            d["bqk"] = np.ascontiguousarray(np.concatenate(
                [b_qkv[0:C][sl], b_qkv[C:2 * C][sl]])).astype(BF16)
            d["bv"] = np.ascontiguousarray(b_qkv[2 * C:3 * C][sl]).astype(BF16)
        parts.append(d)
    return [dict(parts[core % 2], xt=xts[core // 2]) for core in range(N_CORES)]


def kernel(x, w_qkv, b_qkv, w_proj, b_proj):
    x = np.asarray(x, dtype=np.float32)
    w_qkv = np.asarray(w_qkv, dtype=np.float32)
    b_qkv = np.asarray(b_qkv, dtype=np.float32)
    w_proj = np.asarray(w_proj, dtype=np.float32)
    b_proj = np.asarray(b_proj, dtype=np.float32)

    use_bias = bool(np.any(b_qkv))
    nc = _get_nc(use_bias)
    in_maps = _make_in_maps(x, w_qkv, b_qkv, w_proj, use_bias)

    res = run_bass_kernel_spmd(nc, in_maps, list(range(N_CORES)))
    y = np.empty((B, T, C), dtype=np.float32)
    for b in range(B):
        y[b] = res.results[2 * b]["y"] + res.results[2 * b + 1]["y"]
    if np.any(b_proj):
        y += b_proj[None, None, :]
    return y


# revision 17
# speedup vs baseline: 1277.6792x; 2.1778x over previous
"""Causal self-attention on 8 TRN2 NeuronCores.

Sharding: core_id = 2*b + g  (b = batch 0..3, g = head-group 0..1, 8 heads each).
Each core computes qkv for its 8 heads, causal flash-style attention, and a
partial projection (its 512 channels x full w_proj rows). Host sums the two
partials per batch.

Layout strategy (everything transposed so no on-device transposes are needed):
  - x^T [C, T] per batch (host pre-transposed, bf16)
  - Q^T, K^T computed as W^T @ x^T  -> [512, 2048] (channel on partitions)
  - V computed directly as x @ W_v  -> [2048, 512] (token on partitions),
    stored with a ones column per head (V' = [V_h | 1]) so the attention AV
    matmul also produces the softmax row-sums.
  - S^T = K @ Q^T per 128-token j-chunk, both heads of a pair row-tiled into
    one PSUM tile; exp on ACT; causal zeroing via affine_select on GPSIMD.
  - O^T accumulated in PSUM, normalized with reciprocal+partition_broadcast.
  - proj consumes O^T as the stationary matmul operand.
"""

import numpy as np
import ml_dtypes

import concourse.bass as bass
import concourse.tile as tile
from concourse import bacc, mybir
from concourse.bass_utils import run_bass_kernel_spmd

BF16 = ml_dtypes.bfloat16

B, T, C = 4, 2048, 1024
H = 16               # total heads
D = C // H           # 64
HG = 8               # heads per core (head-group)
CL = HG * D          # 512 local channels
N_CORES = 8
SCALE = 1.0 / float(np.sqrt(D))

NCC = C // 128       # 8 c-chunks
NT4 = T // 512       # 4 t-tiles of 512
NT16 = T // 128      # 16 t-chunks of 128

_CACHE = {}


def _emit_body(nc, pools, tensors, use_bias, rep):
    dt = mybir.dt
    psum_s, psum_o, pwork, ywork, norm = pools
    (xt, wqk, wv, wp, qkt, vps, otp, y_d, ones_row, bqk_sb, bv_sb) = tensors

    # ---- phase 1a helper: Q^T or K^T for one 128-channel chunk ----
    def emit_qk(mc, tts=range(NT4)):
        for tt in tts:
            ps = psum_s.tile([128, 512], dt.float32, tag="s",
                              name=f"mm_qk_{rep}_{mc}_{tt}")
            for cc in range(NCC):
                nc.tensor.matmul(
                    ps[:],
                    wqk[cc][:, mc * 128:(mc + 1) * 128],
                    xt[cc][:, tt * 512:(tt + 1) * 512],
                    start=(cc == 0), stop=(cc == NCC - 1 and not use_bias),
                )
            if use_bias:
                nc.tensor.matmul(
                    ps[:],
                    bqk_sb[:, mc * 128:(mc + 1) * 128],
                    ones_row[:, tt * 512:(tt + 1) * 512],
                    start=False, stop=True,
                )
            nc.vector.tensor_copy(qkt[mc][:, tt * 512:(tt + 1) * 512], ps[:])

    def emit_v(t16):
        ps = psum_s.tile([128, 512], dt.float32, tag="s", name=f"mm_v_{rep}_{t16}")
        for cc in range(NCC):
            nc.tensor.matmul(
                ps[:],
                xt[cc][:, t16 * 128:(t16 + 1) * 128],
                wv[cc][:],
                start=(cc == 0), stop=(cc == NCC - 1 and not use_bias),
            )
        if use_bias:
            nc.tensor.matmul(
                ps[:],
                ones_row[:, t16 * 128:(t16 + 1) * 128],
                bv_sb[:],
                start=False, stop=True,
            )
        vt = vps[t16]
        nc.vector.tensor_copy(
            vt[:].rearrange("p (h e) -> p h e", e=D + 1)[:, :, 0:D],
            ps[:].rearrange("p (h d) -> p h d", d=D),
        )

    def emit_attn(hp, qt4):
        q0 = qt4 * 512
        nj = 4 * (qt4 + 1)
        qts, kts = qkt[hp], qkt[4 + hp]
        o_ps = []
        for hi in range(2):
            o_ps.append(psum_o.tile([D + 1, 512], dt.float32, tag="o", bufs=2,
                                    name=f"o_{rep}_{qt4}_{hp}_{hi}"))
        for jc in range(nj):
            j0 = jc * 128
            off = j0 - q0
            # diagonal blocks: only columns q >= j0 can be unmasked
            c0 = max(0, off)        # first useful column in this q-tile
            w = 512 - c0            # columns computed
            s_pair = psum_s.tile([128, 1024], dt.float32, tag="s",
                                 name=f"s_{rep}_{qt4}_{hp}_{jc}")
            for hi in range(2):
                nc.tensor.matmul(
                    s_pair[:, hi * 512 + c0:(hi + 1) * 512],
                    kts[hi * D:(hi + 1) * D, j0:j0 + 128],
                    qts[hi * D:(hi + 1) * D, q0 + c0:q0 + 512],
                    start=True, stop=True,
                )
            p_pair = pwork.tile([128, 1024], dt.bfloat16, tag="p",
                                name=f"p_{rep}_{qt4}_{hp}_{jc}")
            pv = p_pair[:].rearrange("p (h q) -> p h q", h=2)[:, :, c0:512]
            nc.scalar.activation(
                pv,
                s_pair[:].rearrange("p (h q) -> p h q", h=2)[:, :, c0:512],
                mybir.ActivationFunctionType.Exp, scale=SCALE)
            if off > -128:
                # keep where q_global >= j_global; in the clipped view the
                # column index is qi' = qi - c0, so keep iff qi' >= jj.
                nc.gpsimd.affine_select(
                    out=pv, in_=pv,
                    compare_op=mybir.AluOpType.is_ge,
                    fill=0.0, base=0,
                    pattern=[[0, 2], [1, w]],
                    channel_multiplier=-1,
                )
            for hi in range(2):
                h = 2 * hp + hi
                nc.tensor.matmul(
                    o_ps[hi][:, c0:512],
                    vps[jc][:, h * (D + 1):(h + 1) * (D + 1)],
                    p_pair[:, hi * 512 + c0:(hi + 1) * 512],
                    start=(jc == 0), stop=(jc == nj - 1),
                )
        for hi in range(2):
            recip = norm.tile([1, 512], dt.float32, tag="recip",
                              name=f"recip_{rep}_{qt4}_{hp}_{hi}")
            nc.vector.reciprocal(recip[:], o_ps[hi][D:D + 1, :])
            bcast = norm.tile([D, 512], dt.float32, tag="bcast",
                              name=f"bcast_{rep}_{qt4}_{hp}_{hi}")
            nc.gpsimd.partition_broadcast(bcast[:], recip[:])
            nc.vector.tensor_mul(
                otp[hp][hi * D:(hi + 1) * D, q0:q0 + 512],
                o_ps[hi][0:D, :], bcast[:])

    def emit_proj(qt4):
        for qc in range(qt4 * 4, qt4 * 4 + 4):
            for nt in range(2):
                ps = psum_s.tile([128, 512], dt.float32, tag="s",
                                  name=f"mm_y_{rep}_{qc}_{nt}")
                for hp in range(4):
                    nc.tensor.matmul(
                        ps[:],
                        otp[hp][:, qc * 128:(qc + 1) * 128],
                        wp[hp][:, nt * 512:(nt + 1) * 512],
                        start=(hp == 0), stop=(hp == 3),
                    )
                y_sb = ywork.tile([128, 512], dt.float32, tag="y",
                                  name=f"y_{rep}_{qc}_{nt}")
                nc.vector.tensor_copy(y_sb[:], ps[:])
                nc.sync.dma_start(
                    y_d[qc * 128:(qc + 1) * 128, nt * 512:(nt + 1) * 512],
                    y_sb[:])

    # Emission order tuned for overlap: get head-pair 0 ready fast so the
    # ACT-bound attention starts early, then stream the remaining QKV work
    # into PE gaps. Proj for a q-range is emitted as soon as the last
    # head-pair has produced it (attention loops qt4-major inside hp).
    # Lead-in: Q,K (first t-tile) for ALL head-pairs + V for the first
    # q-tile, then sweep q-tiles with head-pairs inner — every attention
    # unit's dependencies are ready well before ACT reaches it, and the
    # next t-tile's QKV streams into PE gaps during each sweep.
    for mc in range(8):
        emit_qk(mc, [0])
    for t16 in range(4):
        emit_v(t16)
    for qt4 in range(NT4):
        for hp in range(4):
            emit_attn(hp, qt4)
            if qt4 < NT4 - 1:
                # prefetch next q-tile's QKV in small per-unit chunks
                emit_qk(2 * hp, [qt4 + 1])
                emit_qk(2 * hp + 1, [qt4 + 1])
                if hp < 2:
                    emit_v(4 * qt4 + 4 + 2 * hp)
                    emit_v(4 * qt4 + 5 + 2 * hp)
        emit_proj(qt4)


def _build(use_bias: bool, reps: int = 1):
    nc = bacc.Bacc("TRN2", target_bir_lowering=False, debug=False,
                   num_devices=N_CORES)
    dt = mybir.dt

    xt_d = nc.dram_tensor("xt", [C, T], dt.bfloat16, kind="ExternalInput").ap()
    wqk_d = nc.dram_tensor("wqk", [C, 2 * CL], dt.bfloat16, kind="ExternalInput").ap()
    wv_d = nc.dram_tensor("wv", [C, CL], dt.bfloat16, kind="ExternalInput").ap()
    wp_d = nc.dram_tensor("wp", [CL, C], dt.bfloat16, kind="ExternalInput").ap()
    bqk_d = bv_d = None
    if use_bias:
        bqk_d = nc.dram_tensor("bqk", [2 * CL], dt.bfloat16, kind="ExternalInput").ap()
        bv_d = nc.dram_tensor("bv", [CL], dt.bfloat16, kind="ExternalInput").ap()
    y_d = nc.dram_tensor("y", [T, C], dt.float32, kind="ExternalOutput").ap()

    with tile.TileContext(nc) as tc:
        with (
            tc.tile_pool(name="const", bufs=1) as const,
            tc.tile_pool(name="psum_s", bufs=3, space="PSUM") as psum_s,
            tc.tile_pool(name="psum_o", bufs=2, space="PSUM") as psum_o,
            tc.tile_pool(name="pwork", bufs=8) as pwork,
            tc.tile_pool(name="ywork", bufs=4) as ywork,
            tc.tile_pool(name="norm", bufs=6) as norm,
        ):
            # ---- persistent SBUF inputs ----
            # Lead-in DMAs first: the slices attention q-tile 0 needs, so
            # compute starts ~4us in instead of after the full ~8MB load.
            xt = [const.tile([128, T], dt.bfloat16, tag=f"xt{cc}", name=f"xt{cc}")
                  for cc in range(NCC)]
            wqk = [const.tile([128, 2 * CL], dt.bfloat16, tag=f"wqk{cc}",
                              name=f"wqk{cc}") for cc in range(NCC)]
            wv = [const.tile([128, CL], dt.bfloat16, tag=f"wv{cc}", name=f"wv{cc}")
                  for cc in range(NCC)]
            wp = [const.tile([128, C], dt.bfloat16, tag=f"wp{hp}", name=f"wp{hp}")
                  for hp in range(4)]
            for cc in range(NCC):
                r = slice(cc * 128, (cc + 1) * 128)
                nc.sync.dma_start(xt[cc][:, 0:512], xt_d[r, 0:512])
                nc.sync.dma_start(wqk[cc][:, 0:128], wqk_d[r, 0:128])
                nc.sync.dma_start(wqk[cc][:, CL:CL + 128], wqk_d[r, CL:CL + 128])
                nc.sync.dma_start(wv[cc][:], wv_d[r, :])
            for cc in range(NCC):
                r = slice(cc * 128, (cc + 1) * 128)
                nc.sync.dma_start(xt[cc][:, 512:T], xt_d[r, 512:T])
                nc.sync.dma_start(wqk[cc][:, 128:CL], wqk_d[r, 128:CL])
                nc.sync.dma_start(wqk[cc][:, CL + 128:2 * CL],
                                  wqk_d[r, CL + 128:2 * CL])
            for hp in range(4):
                nc.sync.dma_start(wp[hp][:], wp_d[hp * 128:(hp + 1) * 128, :])
            ones_row = bqk_sb = bv_sb = None
            if use_bias:
                ones_row = const.tile([1, T], dt.bfloat16, tag="ones_row",
                                      name="ones_row")
                nc.vector.memset(ones_row[:], 1.0)
                bqk_sb = const.tile([1, 2 * CL], dt.bfloat16, tag="bqk", name="bqk_sb")
                nc.sync.dma_start(bqk_sb[:], bqk_d[:].rearrange("n -> 1 n"))
                bv_sb = const.tile([1, CL], dt.bfloat16, tag="bv", name="bv_sb")
                nc.sync.dma_start(bv_sb[:], bv_d[:].rearrange("n -> 1 n"))

            # persistent intermediate tensors
            qkt = []   # 8 tiles [128, T]: 0..3 = Q^T head-pairs, 4..7 = K^T
            for i in range(8):
                qkt.append(const.tile([128, T], dt.bfloat16, tag=f"qkt{i}",
                                      name=f"qkt{i}"))
            vps = []   # 16 tiles [128, 8*65]: V' per t-chunk
            for i in range(NT16):
                vt = const.tile([128, HG * (D + 1)], dt.bfloat16,
                                tag=f"vp{i}", name=f"vp{i}")
                # ones column per head (col 64 of each 65-wide group)
                nc.vector.memset(
                    vt[:].rearrange("p (h e) -> p h e", e=D + 1)[:, :, D:D + 1], 1.0)
                vps.append(vt)
            otp = []   # 4 tiles [128, T]: O^T head-pairs
            for hp in range(4):
                otp.append(const.tile([128, T], dt.bfloat16, tag=f"otp{hp}",
                                      name=f"otp{hp}"))

            pools = (psum_s, psum_o, pwork, ywork, norm)
            tensors = (xt, wqk, wv, wp, qkt, vps, otp, y_d, ones_row, bqk_sb, bv_sb)
            for rep in range(reps):
                _emit_body(nc, pools, tensors, use_bias, rep)

    nc.compile()
    return nc


def _get_nc(use_bias: bool, reps: int = 1):
    key = (use_bias, reps)
    if key not in _CACHE:
        _CACHE[key] = _build(use_bias, reps)
    return _CACHE[key]


def _make_in_maps(x, w_qkv, b_qkv, w_proj, use_bias):
    xts = [np.ascontiguousarray(x[b].T).astype(BF16) for b in range(B)]
    parts = []
    for g in range(2):
        sl = slice(g * CL, (g + 1) * CL)
        wqk = np.ascontiguousarray(np.concatenate(
            [w_qkv[:, 0:C][:, sl], w_qkv[:, C:2 * C][:, sl]], axis=1)).astype(BF16)
        wv = np.ascontiguousarray(w_qkv[:, 2 * C:3 * C][:, sl]).astype(BF16)
        wp = np.ascontiguousarray(w_proj[sl, :]).astype(BF16)
        d = {"wqk": wqk, "wv": wv, "wp": wp}
        if use_bias:
            d["bqk"] = np.ascontiguousarray(np.concatenate(
                [b_qkv[0:C][sl], b_qkv[C:2 * C][sl]])).astype(BF16)
            d["bv"] = np.ascontiguousarray(b_qkv[2 * C:3 * C][sl]).astype(BF16)
        parts.append(d)
    return [dict(parts[core % 2], xt=xts[core // 2]) for core in range(N_CORES)]


def kernel(x, w_qkv, b_qkv, w_proj, b_proj):
    x = np.asarray(x, dtype=np.float32)
    w_qkv = np.asarray(w_qkv, dtype=np.float32)
    b_qkv = np.asarray(b_qkv, dtype=np.float32)
    w_proj = np.asarray(w_proj, dtype=np.float32)
    b_proj = np.asarray(b_proj, dtype=np.float32)

    use_bias = bool(np.any(b_qkv))
    nc = _get_nc(use_bias)
    in_maps = _make_in_maps(x, w_qkv, b_qkv, w_proj, use_bias)

    res = run_bass_kernel_spmd(nc, in_maps, list(range(N_CORES)))
    y = np.empty((B, T, C), dtype=np.float32)
    for b in range(B):
        y[b] = res.results[2 * b]["y"] + res.results[2 * b + 1]["y"]
    if np.any(b_proj):
        y += b_proj[None, None, :]
    return y


# revision 19
# speedup vs baseline: 1355.7609x; 1.0611x over previous
"""Causal self-attention on 8 TRN2 NeuronCores.

Sharding: core_id = 2*b + g  (b = batch 0..3, g = head-group 0..1, 8 heads each).
Each core computes qkv for its 8 heads, causal flash-style attention, and a
partial projection (its 512 channels x full w_proj rows). Host sums the two
partials per batch.

Layout strategy (everything transposed so no on-device transposes are needed):
  - x^T [C, T] per batch (host pre-transposed, bf16)
  - Q^T, K^T computed as W^T @ x^T  -> [512, 2048] (channel on partitions)
  - V computed directly as x @ W_v  -> [2048, 512] (token on partitions),
    stored with a ones column per head (V' = [V_h | 1]) so the attention AV
    matmul also produces the softmax row-sums.
  - S^T = K @ Q^T per 128-token j-chunk, both heads of a pair row-tiled into
    one PSUM tile; exp on ACT; causal zeroing via affine_select on GPSIMD.
  - O^T accumulated in PSUM, normalized with reciprocal+partition_broadcast.
  - proj consumes O^T as the stationary matmul operand.
"""

import numpy as np
import ml_dtypes

import concourse.bass as bass
import concourse.tile as tile
from concourse import bacc, mybir
from concourse.bass_utils import run_bass_kernel_spmd

BF16 = ml_dtypes.bfloat16

B, T, C = 4, 2048, 1024
H = 16               # total heads
D = C // H           # 64
HG = 8               # heads per core (head-group)
CL = HG * D          # 512 local channels
N_CORES = 8
SCALE = 1.0 / float(np.sqrt(D))

NCC = C // 128       # 8 c-chunks
NT4 = T // 512       # 4 t-tiles of 512
NT16 = T // 128      # 16 t-chunks of 128

_CACHE = {}


def _emit_body(nc, pools, tensors, use_bias, rep):
    dt = mybir.dt
    psum_s, psum_o, pwork, ywork, norm = pools
    (xt, wqk, wv, wp, qkt, vps, otp, y_d, ones_row, bqk_sb, bv_sb) = tensors

    # ---- phase 1a helper: Q^T or K^T for one 128-channel chunk ----
    def emit_qk(mc, tts=range(NT4)):
        for tt in tts:
            ps = psum_s.tile([128, 512], dt.float32, tag="s",
                              name=f"mm_qk_{rep}_{mc}_{tt}")
            for cc in range(NCC):
                nc.tensor.matmul(
                    ps[:],
                    wqk[cc][:, mc * 128:(mc + 1) * 128],
                    xt[cc][:, tt * 512:(tt + 1) * 512],
                    start=(cc == 0), stop=(cc == NCC - 1 and not use_bias),
                )
            if use_bias:
                nc.tensor.matmul(
                    ps[:],
                    bqk_sb[:, mc * 128:(mc + 1) * 128],
                    ones_row[:, tt * 512:(tt + 1) * 512],
                    start=False, stop=True,
                )
            nc.vector.tensor_copy(qkt[mc][:, tt * 512:(tt + 1) * 512], ps[:])

    def emit_v(t16):
        ps = psum_s.tile([128, 512], dt.float32, tag="s", name=f"mm_v_{rep}_{t16}")
        for cc in range(NCC):
            nc.tensor.matmul(
                ps[:],
                xt[cc][:, t16 * 128:(t16 + 1) * 128],
                wv[cc][:],
                start=(cc == 0), stop=(cc == NCC - 1 and not use_bias),
            )
        if use_bias:
            nc.tensor.matmul(
                ps[:],
                ones_row[:, t16 * 128:(t16 + 1) * 128],
                bv_sb[:],
                start=False, stop=True,
            )
        vt = vps[t16]
        nc.vector.tensor_copy(
            vt[:].rearrange("p (h e) -> p h e", e=D + 1)[:, :, 0:D],
            ps[:].rearrange("p (h d) -> p h d", d=D),
        )

    def emit_attn(hp, qt4):
        q0 = qt4 * 512
        nj = 4 * (qt4 + 1)
        qts, kts = qkt[hp], qkt[4 + hp]
        o_ps = []
        for hi in range(2):
            o_ps.append(psum_o.tile([D + 1, 512], dt.float32, tag="o", bufs=2,
                                    name=f"o_{rep}_{qt4}_{hp}_{hi}"))
        for jc in range(nj):
            j0 = jc * 128
            off = j0 - q0
            # diagonal blocks: only columns q >= j0 can be unmasked
            c0 = max(0, off)        # first useful column in this q-tile
            w = 512 - c0            # columns computed
            s_pair = psum_s.tile([128, 1024], dt.float32, tag="s",
                                 name=f"s_{rep}_{qt4}_{hp}_{jc}")
            for hi in range(2):
                nc.tensor.matmul(
                    s_pair[:, hi * 512 + c0:(hi + 1) * 512],
                    kts[hi * D:(hi + 1) * D, j0:j0 + 128],
                    qts[hi * D:(hi + 1) * D, q0 + c0:q0 + 512],
                    start=True, stop=True,
                )
            p_pair = pwork.tile([128, 1024], dt.bfloat16, tag="p",
                                name=f"p_{rep}_{qt4}_{hp}_{jc}")
            pv = p_pair[:].rearrange("p (h q) -> p h q", h=2)[:, :, c0:512]
            nc.scalar.activation(
                pv,
                s_pair[:].rearrange("p (h q) -> p h q", h=2)[:, :, c0:512],
                mybir.ActivationFunctionType.Exp, scale=SCALE)
            if off > -128:
                # keep where q_global >= j_global; in the clipped view the
                # column index is qi' = qi - c0, so keep iff qi' >= jj.
                nc.gpsimd.affine_select(
                    out=pv, in_=pv,
                    compare_op=mybir.AluOpType.is_ge,
                    fill=0.0, base=0,
                    pattern=[[0, 2], [1, w]],
                    channel_multiplier=-1,
                )
            for hi in range(2):
                h = 2 * hp + hi
                nc.tensor.matmul(
                    o_ps[hi][:, c0:512],
                    vps[jc][:, h * (D + 1):(h + 1) * (D + 1)],
                    p_pair[:, hi * 512 + c0:(hi + 1) * 512],
                    start=(jc == 0), stop=(jc == nj - 1),
                )
        for hi in range(2):
            recip = norm.tile([1, 512], dt.float32, tag="recip",
                              name=f"recip_{rep}_{qt4}_{hp}_{hi}")
            nc.vector.reciprocal(recip[:], o_ps[hi][D:D + 1, :])
            bcast = norm.tile([D, 512], dt.float32, tag="bcast",
                              name=f"bcast_{rep}_{qt4}_{hp}_{hi}")
            nc.gpsimd.partition_broadcast(bcast[:], recip[:])
            nc.vector.tensor_mul(
                otp[hp][hi * D:(hi + 1) * D, q0:q0 + 512],
                o_ps[hi][0:D, :], bcast[:])

    def emit_proj(qt4, qcs=None):
        for qc in (qcs if qcs is not None else range(qt4 * 4, qt4 * 4 + 4)):
            for nt in range(2):
                ps = psum_s.tile([128, 512], dt.float32, tag="s",
                                  name=f"mm_y_{rep}_{qc}_{nt}")
                for hp in range(4):
                    nc.tensor.matmul(
                        ps[:],
                        otp[hp][:, qc * 128:(qc + 1) * 128],
                        wp[hp][:, nt * 512:(nt + 1) * 512],
                        start=(hp == 0), stop=(hp == 3),
                    )
                y_sb = ywork.tile([128, 512], dt.float32, tag="y",
                                  name=f"y_{rep}_{qc}_{nt}")
                nc.vector.tensor_copy(y_sb[:], ps[:])
                nc.sync.dma_start(
                    y_d[qc * 128:(qc + 1) * 128, nt * 512:(nt + 1) * 512],
                    y_sb[:])

    # Emission order tuned for overlap: get head-pair 0 ready fast so the
    # ACT-bound attention starts early, then stream the remaining QKV work
    # into PE gaps. Proj for a q-range is emitted as soon as the last
    # head-pair has produced it (attention loops qt4-major inside hp).
    # Lead-in: Q,K (first t-tile) for ALL head-pairs + V for the first
    # q-tile, then sweep q-tiles with head-pairs inner — every attention
    # unit's dependencies are ready well before ACT reaches it, and the
    # next t-tile's QKV streams into PE gaps during each sweep.
    for mc in range(8):
        emit_qk(mc, [0])
    for t16 in range(4):
        emit_v(t16)
    for qt4 in range(NT4):
        for hp in range(4):
            emit_attn(hp, qt4)
            if qt4 < NT4 - 1:
                # prefetch next q-tile's QKV in small per-unit chunks
                emit_qk(2 * hp, [qt4 + 1])
                emit_qk(2 * hp + 1, [qt4 + 1])
                if hp < 2:
                    emit_v(4 * qt4 + 4 + 2 * hp)
                    emit_v(4 * qt4 + 5 + 2 * hp)
            # previous q-tile's proj, one q-chunk per unit: fills PE gaps
            # without a burst that outranks this sweep's S matmuls
            if qt4 > 0:
                emit_proj(qt4 - 1, [4 * (qt4 - 1) + hp])
    emit_proj(NT4 - 1)


def _build(use_bias: bool, reps: int = 1):
    nc = bacc.Bacc("TRN2", target_bir_lowering=False, debug=False,
                   num_devices=N_CORES)
    dt = mybir.dt

    xt_d = nc.dram_tensor("xt", [C, T], dt.bfloat16, kind="ExternalInput").ap()
    wqk_d = nc.dram_tensor("wqk", [C, 2 * CL], dt.bfloat16, kind="ExternalInput").ap()
    wv_d = nc.dram_tensor("wv", [C, CL], dt.bfloat16, kind="ExternalInput").ap()
    wp_d = nc.dram_tensor("wp", [CL, C], dt.bfloat16, kind="ExternalInput").ap()
    bqk_d = bv_d = None
    if use_bias:
        bqk_d = nc.dram_tensor("bqk", [2 * CL], dt.bfloat16, kind="ExternalInput").ap()
        bv_d = nc.dram_tensor("bv", [CL], dt.bfloat16, kind="ExternalInput").ap()
    y_d = nc.dram_tensor("y", [T, C], dt.float32, kind="ExternalOutput").ap()

    with tile.TileContext(nc) as tc:
        with (
            tc.tile_pool(name="const", bufs=1) as const,
            tc.tile_pool(name="psum_s", bufs=3, space="PSUM") as psum_s,
            tc.tile_pool(name="psum_o", bufs=2, space="PSUM") as psum_o,
            tc.tile_pool(name="pwork", bufs=8) as pwork,
            tc.tile_pool(name="ywork", bufs=6) as ywork,
            tc.tile_pool(name="norm", bufs=8) as norm,
        ):
            # ---- persistent SBUF inputs ----
            # Lead-in DMAs first: the slices attention q-tile 0 needs, so
            # compute starts ~4us in instead of after the full ~8MB load.
            xt = [const.tile([128, T], dt.bfloat16, tag=f"xt{cc}", name=f"xt{cc}")
                  for cc in range(NCC)]
            wqk = [const.tile([128, 2 * CL], dt.bfloat16, tag=f"wqk{cc}",
                              name=f"wqk{cc}") for cc in range(NCC)]
            wv = [const.tile([128, CL], dt.bfloat16, tag=f"wv{cc}", name=f"wv{cc}")
                  for cc in range(NCC)]
            wp = [const.tile([128, C], dt.bfloat16, tag=f"wp{hp}", name=f"wp{hp}")
                  for hp in range(4)]
            for cc in range(NCC):
                r = slice(cc * 128, (cc + 1) * 128)
                nc.sync.dma_start(xt[cc][:, 0:512], xt_d[r, 0:512])
                nc.sync.dma_start(wqk[cc][:, 0:128], wqk_d[r, 0:128])
                nc.sync.dma_start(wqk[cc][:, CL:CL + 128], wqk_d[r, CL:CL + 128])
                nc.sync.dma_start(wv[cc][:], wv_d[r, :])
            for cc in range(NCC):
                r = slice(cc * 128, (cc + 1) * 128)
                nc.sync.dma_start(xt[cc][:, 512:T], xt_d[r, 512:T])
                nc.sync.dma_start(wqk[cc][:, 128:CL], wqk_d[r, 128:CL])
                nc.sync.dma_start(wqk[cc][:, CL + 128:2 * CL],
                                  wqk_d[r, CL + 128:2 * CL])
            for hp in range(4):
                nc.sync.dma_start(wp[hp][:], wp_d[hp * 128:(hp + 1) * 128, :])
            ones_row = bqk_sb = bv_sb = None
            if use_bias:
                ones_row = const.tile([1, T], dt.bfloat16, tag="ones_row",
                                      name="ones_row")
                nc.vector.memset(ones_row[:], 1.0)
                bqk_sb = const.tile([1, 2 * CL], dt.bfloat16, tag="bqk", name="bqk_sb")
                nc.sync.dma_start(bqk_sb[:], bqk_d[:].rearrange("n -> 1 n"))
                bv_sb = const.tile([1, CL], dt.bfloat16, tag="bv", name="bv_sb")
                nc.sync.dma_start(bv_sb[:], bv_d[:].rearrange("n -> 1 n"))

            # persistent intermediate tensors
            qkt = []   # 8 tiles [128, T]: 0..3 = Q^T head-pairs, 4..7 = K^T
            for i in range(8):
                qkt.append(const.tile([128, T], dt.bfloat16, tag=f"qkt{i}",
                                      name=f"qkt{i}"))
            vps = []   # 16 tiles [128, 8*65]: V' per t-chunk
            for i in range(NT16):
                vt = const.tile([128, HG * (D + 1)], dt.bfloat16,
                                tag=f"vp{i}", name=f"vp{i}")
                # ones column per head (col 64 of each 65-wide group)
                nc.vector.memset(
                    vt[:].rearrange("p (h e) -> p h e", e=D + 1)[:, :, D:D + 1], 1.0)
                vps.append(vt)
            otp = []   # 4 tiles [128, T]: O^T head-pairs
            for hp in range(4):
                otp.append(const.tile([128, T], dt.bfloat16, tag=f"otp{hp}",
                                      name=f"otp{hp}"))

            pools = (psum_s, psum_o, pwork, ywork, norm)
            tensors = (xt, wqk, wv, wp, qkt, vps, otp, y_d, ones_row, bqk_sb, bv_sb)
            for rep in range(reps):
                _emit_body(nc, pools, tensors, use_bias, rep)

    nc.compile()
    return nc


def _get_nc(use_bias: bool, reps: int = 1):
    key = (use_bias, reps)
    if key not in _CACHE:
        _CACHE[key] = _build(use_bias, reps)
    return _CACHE[key]


def _make_in_maps(x, w_qkv, b_qkv, w_proj, use_bias):
    xts = [np.ascontiguousarray(x[b].T).astype(BF16) for b in range(B)]
    parts = []
    for g in range(2):
        sl = slice(g * CL, (g + 1) * CL)
        wqk = np.ascontiguousarray(np.concatenate(
            [w_qkv[:, 0:C][:, sl], w_qkv[:, C:2 * C][:, sl]], axis=1)).astype(BF16)
        wv = np.ascontiguousarray(w_qkv[:, 2 * C:3 * C][:, sl]).astype(BF16)
        wp = np.ascontiguousarray(w_proj[sl, :]).astype(BF16)
        d = {"wqk": wqk, "wv": wv, "wp": wp}
        if use_bias:
            d["bqk"] = np.ascontiguousarray(np.concatenate(
                [b_qkv[0:C][sl], b_qkv[C:2 * C][sl]])).astype(BF16)
            d["bv"] = np.ascontiguousarray(b_qkv[2 * C:3 * C][sl]).astype(BF16)
        parts.append(d)
    return [dict(parts[core % 2], xt=xts[core // 2]) for core in range(N_CORES)]


def kernel(x, w_qkv, b_qkv, w_proj, b_proj):
    x = np.asarray(x, dtype=np.float32)
    w_qkv = np.asarray(w_qkv, dtype=np.float32)
    b_qkv = np.asarray(b_qkv, dtype=np.float32)
    w_proj = np.asarray(w_proj, dtype=np.float32)
    b_proj = np.asarray(b_proj, dtype=np.float32)

    use_bias = bool(np.any(b_qkv))
    nc = _get_nc(use_bias)
    in_maps = _make_in_maps(x, w_qkv, b_qkv, w_proj, use_bias)

    res = run_bass_kernel_spmd(nc, in_maps, list(range(N_CORES)))
    y = np.empty((B, T, C), dtype=np.float32)
    for b in range(B):
        y[b] = res.results[2 * b]["y"] + res.results[2 * b + 1]["y"]
    if np.any(b_proj):
        y += b_proj[None, None, :]
    return y
